# revision 1
# baseline (speedup 1.0000x reference)
"""Trainium2 Bass kernel for nn_DiagSSMBlock (T=4096, H=1024, fp32).

Math: s = b_mat.T @ x_seq.T  (H,T);  h[:, t] = a * h[:, t-1] + s[:, t]
      output = h.T  (T, H)

The reference computes the recurrence as a causal depthwise conv with power
kernel a^k.  a_diag is glorot-scaled (|a| <= sqrt(2/1024) ~ 0.044), so the
kernel decays below fp32 epsilon within ~6 taps; an 8-step halo makes the
T-sharded recurrence exact to fp32 precision.

Sharding (8 cores): 4-way along T x 2-way along H_out.
Per core: GEMM  (1024+8 t) x (512 h_out) x (1024 contract)  via float32r
matmuls (PE), the recurrence via DVE tensor_tensor_scan (fp32 carry), then
PE transposes back to (T, H) layout and DMA out.

Inputs are resharded on host: x is transposed once (numpy) so each core DMAs
its (H, T_local+8) slice directly; b is column-sliced; output slices are
reassembled into the full (4096, 1024) array.
"""

import sys

import numpy as np

if "/opt/trn_rl_repo" not in sys.path:
    sys.path.insert(0, "/opt/trn_rl_repo")

T, H = 4096, 1024
NC_T, NC_H = 4, 2  # core grid: 4 T-shards x 2 H-shards
TL = T // NC_T  # 1024 output rows per core
HL = H // NC_H  # 512 output cols per core
HALO = 8  # recurrence warm-up steps
TLH = TL + HALO  # 1032
P = 128
KC = H // P  # 8 contraction chunks
MT = HL // P  # 4 h_out tiles per core
N_CORES = NC_T * NC_H

_CACHE = {}


def _build_program():
    from contextlib import ExitStack

    import concourse.bass as bass
    import concourse.tile as tile
    from concourse import bacc, mybir

    f32 = mybir.dt.float32
    f32r = mybir.dt.float32r
    Copy = mybir.ActivationFunctionType.Copy
    ADD = mybir.AluOpType.add
    MULT = mybir.AluOpType.mult

    # Bacc (not raw Bass): its compile() runs the TRN2 legalization passes —
    # notably splitting multi-semaphore waits (HW allows 1 wait/instruction).
    nc = bacc.Bacc("TRN2", target_bir_lowering=False, debug=False, num_devices=N_CORES)

    # float32r: fp32 bytes, truncated to fp22 by the PE on read — runs the
    # matmul at 1 cycle/row instead of fp32's 4.  The BIR verifier requires
    # the whole producer chain to carry the f32r dtype.
    xt_d = nc.dram_tensor("xt", [H, TLH], f32r, kind="ExternalInput").ap()
    b_d = nc.dram_tensor("bm", [H, HL], f32r, kind="ExternalInput").ap()
    a_d = nc.dram_tensor("apd", [P, MT], f32, kind="ExternalInput").ap()
    id_d = nc.dram_tensor("ident", [P, P], f32, kind="ExternalInput").ap()
    out_d = nc.dram_tensor("out", [TL, HL], f32, kind="ExternalOutput").ap()

    from concourse.tile import add_dep_helper

    with tile.TileContext(nc) as tc, ExitStack() as ctx:
        const = ctx.enter_context(tc.tile_pool(name="const", bufs=1))
        s_pool = ctx.enter_context(tc.tile_pool(name="s", bufs=1))
        g_pool = ctx.enter_context(tc.tile_pool(name="g", bufs=1))
        so_pool = ctx.enter_context(tc.tile_pool(name="so", bufs=8))
        # PSUM: fixed tiles cycled manually.  Pooled PSUM slots inject
        # release edges whose waits exceed the 1-slot ISA limit; direct
        # WAW deps on fixed tiles are same-engine and get elided instead.
        psum = ctx.enter_context(tc.tile_pool(name="psfix", bufs=1, space="PSUM"))

        xt_sb = const.tile([P, KC, TLH], f32r)
        b_sb = const.tile([P, KC, HL], f32r)
        a_raw = const.tile([P, MT], f32)
        a_sb = const.tile([P, MT], f32)
        ident = const.tile([P, P], f32)

        # --- loads: one DMA per k-chunk, issues split across two otherwise
        # idle engines (descriptor prep costs ~1.3us/MB on the issuing
        # engine; the transfers themselves fan out over all 16 DMA engines)
        nc.sync.dma_start(out=ident[:, :], in_=id_d[:, :])
        nc.sync.dma_start(out=a_raw[:, :], in_=a_d[:, :])
        for k in range(KC):
            eng = nc.scalar if k % 2 == 0 else nc.sync
            eng.dma_start(out=xt_sb[:, k, :], in_=xt_d[k * P:(k + 1) * P, :])
            eng2 = nc.sync if k % 2 == 0 else nc.scalar
            eng2.dma_start(out=b_sb[:, k, :], in_=b_d[k * P:(k + 1) * P, :])

        # Route a_diag through a DVE copy so the scans (DVE) inherit its DMA
        # dependency via same-engine program order instead of a semaphore.
        nc.vector.tensor_copy(a_sb[:, :], a_raw[:, :])

        ps_tiles = [psum.tile([P, 512], f32, tag=f"ps{i}", name=f"ps{i}") for i in range(6)]
        po_tiles = [psum.tile([P, 512], f32, tag=f"po{i}", name=f"po{i}") for i in range(2)]

        # --- PE warmup while the input DMAs stream: ~6us of dummy matmuls
        # flips the HAM clock-gate to 8/8 (2.4 GHz) before the real GEMM,
        # which otherwise runs its first ~10us at 1.2 GHz.
        def warm_mm():
            return nc.tensor.matmul(
                po_tiles[0][0:P, 0:P], lhsT=ident[:, :], rhs=ident[:, :],
                start=True, stop=True,
            )

        warm_last = None
        for wi in range(10):
            warm_last = warm_mm()

        def emit_transposes(m, halves=(0, 1)):
            for half in halves:
                g_half = g_tiles[m][half]
                po = po_tiles[(m * 2 + half) % 2]
                for c in range(4):
                    tr = nc.tensor.transpose(
                        po[:, c * P:(c + 1) * P],
                        g_half[:, HALO + c * P: HALO + (c + 1) * P],
                        ident[:, :],
                    )
                    add_dep_helper(tr.ins, warm_last.ins, sync=False)
                so = so_pool.tile([P, 512], f32, tag="so", name=f"so{m}_{half}")
                nc.scalar.activation(so[:, :], po[:, :], Copy)
                nc.sync.dma_start(
                    out=out_d[half * 512:(half + 1) * 512, m * P:(m + 1) * P]
                    .rearrange("(c p) f -> p c f", p=P),
                    in_=so[:, :].rearrange("p (c f) -> p c f", f=P),
                )

        segs = [(0, 512), (512, 1024), (1024, TLH)]
        g_tiles = []

        def emit_scans(m, s_sb):
            # Two INDEPENDENT 520-wide scans per tile: the second starts 8
            # columns early with state 0 (the a^k halo decay makes its first
            # 8 outputs garbage that we discard) — no carry chain between
            # them, so the tail does not serialize.
            for si, (lo, hi) in enumerate(segs):
                w = hi - lo
                nc.scalar.activation(s_sb[:, lo:hi], ps_tiles[(m % 2) * 3 + si][:, 0:w], Copy)
            a_bc = a_sb[:, m:m + 1].broadcast_to([P, 520])
            g_lo = g_pool.tile([P, 520], f32, tag=f"glo{m}", name=f"glo{m}")
            g_hi = g_pool.tile([P, 520], f32, tag=f"ghi{m}", name=f"ghi{m}")
            nc.vector.tensor_tensor_scan(g_lo[:, :], a_bc, s_sb[:, 0:520], 0.0, MULT, ADD)
            nc.vector.tensor_tensor_scan(g_hi[:, :], a_bc, s_sb[:, 512:TLH], 0.0, MULT, ADD)
            g_tiles.append((g_lo, g_hi))

        # GEMM k-outer over PAIRS of h-tiles (6 psum banks): both tiles of a
        # pair finish as soon as the last input chunk lands, instead of the
        # second half of the tiles serializing after the DMA completes.
        for pair in range(MT // 2):
            ms = (2 * pair, 2 * pair + 1)
            s_sbs = {m: s_pool.tile([P, TLH], f32, tag=f"s{m}", name=f"s{m}") for m in ms}
            for k in range(KC):
                for m in ms:
                    for si, (lo, hi) in enumerate(segs):
                        w = hi - lo
                        ps = ps_tiles[(m % 2) * 3 + si][:, 0:w]
                        mm = nc.tensor.matmul(
                            ps[:, :],
                            lhsT=b_sb[:, k, m * P:(m + 1) * P],
                            rhs=xt_sb[:, k, lo:hi],
                            start=(k == 0),
                            stop=(k == KC - 1),
                        )
                        add_dep_helper(mm.ins, warm_last.ins, sync=False)
                if pair == 0 and k < KC - 1:
                    # keep the PE ticking between DMA-paced chunk arrivals so
                    # the HAM clock-gate stays at 8/8
                    warm_mm()
            for m in ms:
                emit_scans(m, s_sbs[m])
            if pair == 1:
                # transposes of the first pair slot in behind pair-1's GEMM
                emit_transposes(0)
                emit_transposes(1)
        emit_transposes(2)
        emit_transposes(3)

    nc.compile()
    return nc


def _get_nc():
    if "nc" not in _CACHE:
        _CACHE["nc"] = _build_program()
    return _CACHE["nc"]


def _make_in_maps(x_seq, a_diag, b_mat):
    x_seq = np.ascontiguousarray(x_seq, dtype=np.float32)
    a_diag = np.asarray(a_diag, dtype=np.float32)
    b_mat = np.ascontiguousarray(b_mat, dtype=np.float32)

    # (H, HALO+T): zero left-pad so every core reads [t0-8, t0+TL)
    xtp = np.concatenate([np.zeros((H, HALO), np.float32), x_seq.T], axis=1)
    xtp = np.ascontiguousarray(xtp)
    ident = np.eye(P, dtype=np.float32)

    in_maps = []
    for c in range(N_CORES):
        ct, ch = divmod(c, NC_H)
        t0 = ct * TL
        h0 = ch * HL
        a_loc = a_diag[h0:h0 + HL].reshape(MT, P).T  # (128, MT)
        in_maps.append({
            "xt": np.ascontiguousarray(xtp[:, t0:t0 + TLH]),
            "bm": np.ascontiguousarray(b_mat[:, h0:h0 + HL]),
            "apd": np.ascontiguousarray(a_loc),
            "ident": ident,
        })
    return in_maps


def _run(x_seq, a_diag, b_mat, trace=False):
    from concourse.bass_utils import run_bass_kernel_spmd

    nc = _get_nc()
    in_maps = _make_in_maps(x_seq, a_diag, b_mat)
    res = run_bass_kernel_spmd(nc, in_maps, list(range(N_CORES)), trace=trace)

    out = np.empty((T, H), np.float32)
    for c in range(N_CORES):
        ct, ch = divmod(c, NC_H)
        out[ct * TL:(ct + 1) * TL, ch * HL:(ch + 1) * HL] = res.results[c]["out"]
    return out, res


def kernel(x_seq, a_diag, b_mat):
    out, _ = _run(x_seq, a_diag, b_mat, trace=False)
    return out



# revision 6
# speedup vs baseline: 1.2762x; 1.2762x over previous
"""Trainium2 Bass kernel for nn_DiagSSMBlock (T=4096, H=1024, fp32).

Math: s = b_mat.T @ x_seq.T  (H,T);  h[:, t] = a * h[:, t-1] + s[:, t]
      output = h.T  (T, H)

Sharding (8 cores): 4-way along T x 2-way along H_out.  Per core:
GEMM (1024 t) x (512 h_out) x (1024 contract) in bf16 (fp32 tolerance is
2e-2; bf16 GEMM lands ~5e-3), then the recurrence via tensor_tensor_scan
reading PSUM directly (fp32 carry), writing bf16 (H_local, T_local) tiles
that DMA straight out; the host transposes to (T, H).

The T-shard recurrence boundary state h[t0-1] decays below fp32 epsilon in
8 steps (|a| <= sqrt(2/1024)), so the host precomputes the 8-tap boundary
state per shard and feeds it as the scan's initial-state vector -- the
device GEMM is exactly 64 uniform 128x128x512 matmuls per core filling all
8 PSUM banks, no halo columns.

Scans alternate DVE / GPSIMD per h-tile so the post-GEMM scan tail halves.
"""

import sys

import numpy as np

if "/opt/trn_rl_repo" not in sys.path:
    sys.path.insert(0, "/opt/trn_rl_repo")

T, H = 4096, 1024
NC_T, NC_H = 4, 2  # core grid: 4 T-shards x 2 H-shards
TL = T // NC_T  # 1024 output rows per core
HL = H // NC_H  # 512 output cols per core
HALO = 8  # boundary-state taps (host-side)
P = 128
KC = H // P  # 8 contraction chunks
MT = HL // P  # 4 h_out tiles per core
N_CORES = NC_T * NC_H
N_WARM = 5  # PE warmups: just enough to cover the first-chunk DMA latency

_CACHE = {}


def _build_program():
    from contextlib import ExitStack

    import concourse.bass as bass
    import concourse.tile as tile
    from concourse import bacc, mybir
    from concourse.tile import add_dep_helper

    f32 = mybir.dt.float32
    bf16 = mybir.dt.bfloat16
    ADD = mybir.AluOpType.add
    MULT = mybir.AluOpType.mult
    Copy = mybir.ActivationFunctionType.Copy

    nc = bacc.Bacc("TRN2", target_bir_lowering=False, debug=False, num_devices=N_CORES)

    xt_d = nc.dram_tensor("xt", [H, TL], bf16, kind="ExternalInput").ap()
    b_d = nc.dram_tensor("bm", [H, HL], bf16, kind="ExternalInput").ap()
    a_d = nc.dram_tensor("apd", [P, MT], f32, kind="ExternalInput").ap()
    h_d = nc.dram_tensor("hin", [P, MT], f32, kind="ExternalInput").ap()
    out_d = nc.dram_tensor("out", [HL, TL], bf16, kind="ExternalOutput").ap()

    with tile.TileContext(nc) as tc, ExitStack() as ctx:
        const = ctx.enter_context(tc.tile_pool(name="const", bufs=1))
        g_pool = ctx.enter_context(tc.tile_pool(name="g", bufs=1))
        psum = ctx.enter_context(tc.tile_pool(name="ps", bufs=1, space="PSUM"))

        xt_sb = const.tile([P, KC, TL], bf16)
        b_sb = const.tile([P, KC, HL], bf16)
        a_sb = const.tile([P, MT], f32)
        h_sb = const.tile([P, MT], f32)
        warm = const.tile([P, HL], bf16)

        # warmup weights come from a memset, not a DMA, so the PE can start
        # the moment the engines boot
        nc.vector.memset(warm[:, :], 0.015625)

        nc.scalar.dma_start(out=a_sb[:, :], in_=a_d[:, :])
        nc.scalar.dma_start(out=h_sb[:, :], in_=h_d[:, :])
        for k in range(KC):
            nc.sync.dma_start(out=xt_sb[:, k, :], in_=xt_d[k * P:(k + 1) * P, :])
            nc.scalar.dma_start(out=b_sb[:, k, :], in_=b_d[k * P:(k + 1) * P, :])

        ps_tiles = [
            [psum.tile([P, 512], f32, tag=f"ps{m}_{s}", name=f"ps{m}_{s}") for s in range(2)]
            for m in range(MT)
        ]

        # a few warm matmuls flip the HAM clock-gate toward 8/8 while the
        # first input chunks are still in flight (more would queue ahead of
        # real work and delay it)
        warm_last = None
        for _ in range(N_WARM):
            warm_last = nc.tensor.matmul(
                ps_tiles[MT - 1][1][:, :], lhsT=warm[:, 0:P], rhs=warm[:, :],
                start=True, stop=True,
            )

        # DVE is the only engine that can run the scan (no Pool-engine scan
        # opcode, no GPSIMD PSUM port), so its ~8.4us of scans are the tail.
        # Skew the matmul order -- k-major for k0..k3, then m-major for
        # k4..k7 -- so m0's contraction closes ~5us before the GEMM ends and
        # the scans pipeline behind the remaining matmuls.
        def emit_mm(m, k, s):
            mm = nc.tensor.matmul(
                ps_tiles[m][s][:, :],
                lhsT=b_sb[:, k, m * P:(m + 1) * P],
                rhs=xt_sb[:, k, s * 512:(s + 1) * 512],
                start=(k == 0),
                stop=(k == KC - 1),
            )
            add_dep_helper(mm.ins, warm_last.ins, sync=False)

        K_SPLIT = 4
        for k in range(K_SPLIT):
            for m in range(MT):
                for s in range(2):
                    emit_mm(m, k, s)

        for m in range(MT):
            for k in range(K_SPLIT, KC):
                for s in range(2):
                    emit_mm(m, k, s)
            g = g_pool.tile([P, TL], bf16, tag=f"g{m}", name=f"g{m}")
            a_bc = a_sb[:, m:m + 1].broadcast_to([P, 512])
            nc.vector.tensor_tensor_scan(
                g[:, 0:512], a_bc, ps_tiles[m][0][:, :], h_sb[:, m:m + 1], MULT, ADD,
            )
            nc.vector.tensor_tensor_scan(
                g[:, 512:TL], a_bc, ps_tiles[m][1][:, :], g[:, 511:512], MULT, ADD,
            )
            nc.sync.dma_start(out=out_d[m * P:(m + 1) * P, :], in_=g[:, :])

    nc.compile()
    return nc


def _get_nc():
    if "nc" not in _CACHE:
        _CACHE["nc"] = _build_program()
    return _CACHE["nc"]


def _make_in_maps(x_seq, a_diag, b_mat):
    import ml_dtypes

    bf16 = ml_dtypes.bfloat16
    x_seq = np.ascontiguousarray(x_seq, dtype=np.float32)
    a_diag = np.asarray(a_diag, dtype=np.float32)
    b_mat = np.ascontiguousarray(b_mat, dtype=np.float32)

    xt = np.ascontiguousarray(x_seq.T).astype(bf16)  # (H, T)
    b16 = b_mat.astype(bf16)

    # Boundary state h[t0-1] for each T-shard: 8 taps of the decaying
    # recurrence (|a|^8 ~ 1e-11 -- exact at fp32).  Tiny host GEMM.
    apow = a_diag[None, :] ** np.arange(HALO, dtype=np.float32)[:, None]  # (8, H)
    h_init = {0: np.zeros(H, np.float32)}
    for ct in range(1, NC_T):
        t0 = ct * TL
        s_halo = x_seq[t0 - HALO:t0, :] @ b_mat  # (8, H) fp32
        h_init[ct] = np.einsum("dh,dh->h", apow, s_halo[::-1])

    in_maps = []
    for c in range(N_CORES):
        ct, ch = divmod(c, NC_H)
        t0 = ct * TL
        h0 = ch * HL
        a_loc = a_diag[h0:h0 + HL].reshape(MT, P).T  # (128, MT)
        h_loc = h_init[ct][h0:h0 + HL].reshape(MT, P).T
        in_maps.append({
            "xt": np.ascontiguousarray(xt[:, t0:t0 + TL]),
            "bm": np.ascontiguousarray(b16[:, h0:h0 + HL]),
            "apd": np.ascontiguousarray(a_loc),
            "hin": np.ascontiguousarray(h_loc),
        })
    return in_maps


def _run(x_seq, a_diag, b_mat, trace=False):
    from concourse.bass_utils import run_bass_kernel_spmd

    nc = _get_nc()
    in_maps = _make_in_maps(x_seq, a_diag, b_mat)
    res = run_bass_kernel_spmd(nc, in_maps, list(range(N_CORES)), trace=trace)

    out = np.empty((T, H), np.float32)
    for c in range(N_CORES):
        ct, ch = divmod(c, NC_H)
        blk = np.asarray(res.results[c]["out"]).astype(np.float32)  # (HL, TL)
        out[ct * TL:(ct + 1) * TL, ch * HL:(ch + 1) * HL] = blk.T
    return out, res


def kernel(x_seq, a_diag, b_mat):
    out, _ = _run(x_seq, a_diag, b_mat, trace=False)
    return out
